# revision 8
# baseline (speedup 1.0000x reference)
"""3x3 median filter (reflect padding) on Trainium2, v2.5.

Data-parallel over batch (2 images/core on 8 cores). Per core:

Host staging: input cast to bf16, transposed to [BPC, HP=H+2, C, W] with
the two vertical reflect rows pre-staged, then column-DEINTERLEAVED to
[BPC, HP, C, 2, Wh] (par=c%2, j=c//2). Every device-side access pattern
becomes a packed run, which the DVE 2x bf16 mode needs (measured 0.52
ns/elem vs 1.04 otherwise). Output leaves in the same deinterleaved
layout and is merged on host.

Per batch iteration (H=512 output rows = 2 double-steps ds, each a pair
of even/odd row tiles E/O sharing the vertical row-pair reduction; 15
min/max elem-ops per pixel):
  per ds: one 12KB/partition input DMA (3 chunks; A/B chunk first),
          p=min(A,B), q=max(A,B)                        2 TT
          sort3 completion lo/md/hi (E/O stacked,
          stride-0 broadcast of p,q)                    3 TT
  then, stacked across ds*eo*C (strides nest C*3=eo, eo*2=ds, so the
  merged dim fits the 3-dim TensorTensor ISA limit):
          column pairs me_lo/mx_md/mn_md/me_hi          2 TT
          finals x,z,y interior (odd/even stacked)      4 TT
          boundary cols 0, W-1                          6 Act copies
          med3(x,y,z) -> res (per-ds halves)            8 TT
          half-column output DMAs

All min/max on the DVE: the Pool engine's software library rejects
TensorTensor min/max (walrus engine check) and an add/relu decomposition
is numerically unsafe (catastrophic cancellation vs max-rel-err).
Most tiles are bufs=1 — every producer/consumer is on the in-order DVE
queue — only the DMA-facing tiles (in4, res) are double-buffered.
"""

import sys

sys.path.insert(0, "/opt/trn_rl_repo")

import numpy as np
import ml_dtypes

_COMPILED = {}

B, C, H, W = 16, 3, 512, 512
NCORES = 8
BPC = B // NCORES
Wh = W // 2        # 256
HP = H + 2         # padded rows
SR = C * W         # elements per row (1536), same in deinterleaved layout
SBI = HP * SR      # input batch stride
SBO = H * SR       # output batch stride


def _legalize_waits(nc, mybir):
    """Hoist excess sync-waits into a preceding same-engine EventSemaphore.
    The TRN2 ISA allows 1 sync-wait on compute instructions (2 on DMACopy;
    EventSemaphore allows several) but Tile's scheduler can emit more."""
    limits = {"InstEventSemaphore": 2}
    n_hoisted = 0
    for f in nc.m.functions:
        for bb in f.blocks:
            il = bb.instructions
            idx = 0
            while idx < len(il):
                i = il[idx]
                si = i.sync_info
                lim = limits.get(type(i).__name__, 1)
                if si is not None and si.on_wait and len(si.on_wait) > lim:
                    waits = list(si.on_wait)
                    keep, excess = waits[:lim], waits[lim:]
                    hoists = []
                    for j in range(0, len(excess), 2):
                        h = mybir.InstEventSemaphore(
                            name=f"hoistw_{n_hoisted}", ins=[], outs=[])
                        n_hoisted += 1
                        h.engine = i.engine
                        h.sync_info = mybir.SyncInfo(
                            on_wait=excess[j:j + 2], on_update=[])
                        hoists.append(h)
                    i.sync_info = mybir.SyncInfo(
                        on_wait=keep, on_update=si.on_update)
                    for k, h in enumerate(hoists):
                        il.insert(idx + k, h)
                    idx += len(hoists)
                idx += 1
    return n_hoisted


def _build_nc():
    from concourse import bass
    import concourse.mybir as mybir
    from concourse.tile import TileContext

    bf16 = mybir.dt.bfloat16
    MIN = mybir.AluOpType.min
    MAX = mybir.AluOpType.max
    AP = bass.AP

    nc = bass.Bass()
    img = nc.dram_tensor("image", [BPC, HP, C, 2, Wh], bf16,
                         kind="ExternalInput")
    out = nc.dram_tensor("out", [BPC, H, C, 2, Wh], bf16,
                         kind="ExternalOutput")

    def mk(t, off, dims):
        b = t[:]
        return AP(b.tensor, b.offset + off, [list(b.ap[0])] + dims)

    with TileContext(nc) as tc:
        with tc.tile_pool(name="p", bufs=1) as pool:
            for g in range(BPC):
                base = g * SBI
                # tile layouts (per-partition element offsets):
                # in4 [2ds,4t,1536]  t: 0=cE 1=A 2=B 3=cO   off ds*6144+t*1536
                # P   [2ds,2pq,1536]                        off ds*3072+pq*1536
                # lmh [3s,2ds,2eo,C,2,Wh] s: lo,md,hi   off s*6144+ds*3072+eo*1536
                # hp  [4s,2ds,2eo,C,Wh] s: me_lo,mx_md,mn_md,me_hi
                #                                       off s*3072+ds*1536+eo*768
                # xyz [4s,2ds,2eo,C,2,Wh] s: x,y,z,f1   off s*6144+ds*3072+eo*1536
                # res [2ds,2eo,1536]
                in4 = pool.tile([128, 2, 4, C, 2, Wh], bf16, tag="in4", bufs=2)
                P = pool.tile([128, 2, 2, C, 2, Wh], bf16, tag="P")
                lmh = pool.tile([128, 3, 2, 2, C, 2, Wh], bf16, tag="lmh")
                hp = pool.tile([128, 4, 2, 2, C, Wh], bf16, tag="hp")
                xyz = pool.tile([128, 4, 2, 2, C, 2, Wh], bf16, tag="xyz")
                res = pool.tile([128, 2, 2, C, 2, Wh], bf16, tag="res", bufs=2)

                for ds in range(2):
                    r0 = 256 * ds
                    db = ds * 6144
                    # padded rows r0+2j+t per partition; A and B chunks first
                    # (separate queues transfer in parallel) so the P stage
                    # starts as early as possible
                    for t, dr in ((1, 1), (2, 2), (0, 0), (3, 3)):
                        nc.sync.dma_start(out=in4[:, ds, t], in_=AP(
                            img, base + (r0 + dr) * SR,
                            [[2 * SR, 128], [1, 1536]]))

                    # p = min(A,B), q = max(A,B)
                    nc.vector.tensor_tensor(
                        P[:, ds, 0], in4[:, ds, 1], in4[:, ds, 2], MIN)
                    nc.vector.tensor_tensor(
                        P[:, ds, 1], in4[:, ds, 1], in4[:, ds, 2], MAX)

                    # sort3 completion, E/O stacked:
                    # [loE,tE,loO,tO] = min([p,q,p,q],[cE,cE,cO,cO])
                    # [hiE,hiO] = max([q,q],[cE,cO]); md = max(p, t) in place
                    pb = ds * 3072
                    nc.vector.tensor_tensor(
                        mk(lmh, pb, [[1536, 2], [6144, 2], [1, 1536]]),
                        mk(P, pb, [[0, 2], [1536, 2], [1, 1536]]),
                        mk(in4, db, [[4608, 2], [0, 2], [1, 1536]]), MIN)
                    nc.vector.tensor_tensor(
                        mk(lmh, 2 * 6144 + pb, [[1536, 2], [1, 1536]]),
                        mk(P, pb + 1536, [[0, 2], [1, 1536]]),
                        mk(in4, db, [[4608, 2], [1, 1536]]), MAX)
                    md_ap = mk(lmh, 6144 + pb, [[1536, 2], [1, 1536]])
                    nc.vector.tensor_tensor(
                        md_ap, mk(P, pb, [[0, 2], [1, 1536]]), md_ap, MAX)

                # ---- column pairs me[j] = op(v[par0,j], v[par1,j]), stacked
                # over (slot pair, ds*eo*C): (lo,md)->MAX, (md,hi)->MIN
                nc.vector.tensor_tensor(
                    mk(hp, 0, [[3072, 2], [256, 12], [1, 256]]),
                    mk(lmh, 0, [[6144, 2], [512, 12], [1, 256]]),
                    mk(lmh, 256, [[6144, 2], [512, 12], [1, 256]]), MAX)
                nc.vector.tensor_tensor(
                    mk(hp, 2 * 3072, [[3072, 2], [256, 12], [1, 256]]),
                    mk(lmh, 6144, [[6144, 2], [512, 12], [1, 256]]),
                    mk(lmh, 6144 + 256, [[6144, 2], [512, 12], [1, 256]]), MIN)

                # ---- interior finals, odd/even output cols stacked:
                # i=0 -> odd col 2j+1 (par1,j), i=1 -> even col 2j+2
                # (par0,j+1), j=0..254
                o_ap = lambda s: mk(xyz, s * 6144 + 256,
                                    [[512, 12], [-255, 2], [1, 255]])
                me_ap = lambda s: mk(hp, s * 3072,
                                     [[256, 12], [1, 2], [1, 255]])
                th_ap = lambda s: mk(lmh, s * 6144 + 1,
                                     [[512, 12], [255, 2], [1, 255]])
                nc.vector.tensor_tensor(o_ap(0), me_ap(0), th_ap(0), MAX)
                nc.vector.tensor_tensor(o_ap(2), me_ap(3), th_ap(2), MIN)
                nc.vector.tensor_tensor(o_ap(1), me_ap(1), th_ap(1), MIN)
                nc.vector.tensor_tensor(o_ap(1), me_ap(2), o_ap(1), MAX)

                # ---- boundary columns (Act): col0 = even(par0,j0):
                # x=me_lo[0], z=me_hi[0], y=md[par1,0]; col W-1 =
                # odd(par1,j255): x=me_lo[255], z=me_hi[255], y=md[par0,255]
                for dst, src, st in ((0, 0, 0),
                                     (511, 255, 0),
                                     (2 * 6144, 3 * 3072, 0),
                                     (2 * 6144 + 511, 3 * 3072 + 255, 0),
                                     (6144, 6144 + 256, 1),
                                     (6144 + 511, 6144 + 255, 1)):
                    srct, sdim = ((hp, 256), (lmh, 512))[st]
                    nc.scalar.copy(mk(xyz, dst, [[512, 12], [1, 1]]),
                                   mk(srct, src, [[sdim, 12], [1, 1]]))

                # ---- med3(x,y,z) -> res. Batch 0's tail is hidden by batch
                # 1's front, so it uses the cheaper full-batch stacking; the
                # last batch finishes in small per-ds/per-eo pieces so the
                # final output DMAs start as early as possible.
                def med3(hb, n):
                    sl = lambda s: mk(xyz, s * 6144 + hb, [[1, n]])
                    f1, x, y, z = sl(3), sl(0), sl(1), sl(2)
                    nc.vector.tensor_tensor(f1, x, y, MIN)
                    nc.vector.tensor_tensor(x, x, y, MAX)
                    nc.vector.tensor_tensor(x, x, z, MIN)
                    return f1, x

                def res_out(hb, n, ds, eo):
                    for h in range(0, n, 768):
                        nc.sync.dma_start(
                            out=AP(out,
                                   g * SBO + (256 * ds + eo) * SR + h % 1536,
                                   [[2 * SR, 128], [1, 768]]),
                            in_=mk(res, hb + h, [[1, 768]]))

                if g == 0:
                    f1, x = med3(0, 6144)
                    nc.vector.tensor_tensor(
                        mk(res, 0, [[1, 6144]]), f1, x, MAX)
                    for ds in range(2):
                        for eo in range(2):
                            res_out(ds * 3072 + eo * 1536, 1536, ds, eo)
                else:
                    for ds in range(2):
                        hb = ds * 3072
                        f1, x = med3(hb, 3072)
                        for eo in range(2):
                            eb = hb + eo * 1536
                            nc.vector.tensor_tensor(
                                mk(res, eb, [[1, 1536]]),
                                mk(xyz, 3 * 6144 + eb, [[1, 1536]]),
                                mk(xyz, eb, [[1, 1536]]), MAX)
                            res_out(eb, 1536, ds, eo)

    _legalize_waits(nc, mybir)
    return nc


def _stage_input(img_k: np.ndarray) -> np.ndarray:
    """[BPC, C, H, W] f32 -> [BPC, HP, C, 2, Wh] bf16, rows padded
    (reflect), columns deinterleaved."""
    t = img_k.transpose(0, 2, 1, 3)  # [BPC, H, C, W]
    p = np.empty((BPC, HP, C, W), dtype=np.float32)
    p[:, 1:H + 1] = t
    p[:, 0] = t[:, 1]
    p[:, H + 1] = t[:, H - 2]
    q = np.empty((BPC, HP, C, 2, Wh), dtype=ml_dtypes.bfloat16)
    q[..., 0, :] = p[..., 0::2]
    q[..., 1, :] = p[..., 1::2]
    return q


def _unstage_output(o: np.ndarray) -> np.ndarray:
    """[BPC, H, C, 2, Wh] bf16 -> [BPC, C, H, W] f32."""
    full = np.empty((BPC, H, C, W), dtype=np.float32)
    full[..., 0::2] = o[..., 0, :]
    full[..., 1::2] = o[..., 1, :]
    return full.transpose(0, 2, 1, 3)


def kernel(image: np.ndarray) -> np.ndarray:
    from concourse.bass_utils import run_bass_kernel_spmd

    image = np.asarray(image, dtype=np.float32)
    if "nc" not in _COMPILED:
        _COMPILED["nc"] = _build_nc()
    nc = _COMPILED["nc"]

    in_maps = [{"image": _stage_input(image[k * BPC:(k + 1) * BPC])}
               for k in range(NCORES)]
    try:
        res = run_bass_kernel_spmd(nc, in_maps, core_ids=list(range(NCORES)))
    except Exception:
        res = run_bass_kernel_spmd(nc, in_maps, core_ids=list(range(NCORES)))
    return np.concatenate(
        [_unstage_output(np.asarray(res.results[k]["out"]))
         for k in range(NCORES)],
        axis=0)


# revision 9
# speedup vs baseline: 1.0054x; 1.0054x over previous
"""3x3 median filter (reflect padding) on Trainium2, v2.7.

Data-parallel over batch (2 images/core on 8 cores). Per core:

Host staging: input cast to bf16, transposed to [BPC, HP=H+2, C, W] with
the two vertical reflect rows pre-staged, then column-DEINTERLEAVED to
[BPC, HP, C, 2, Wh] (par=c%2, j=c//2). Every device-side access pattern
becomes a packed run, which the DVE 2x bf16 mode needs (measured 0.52
ns/elem vs 1.04 otherwise). Output leaves in the same deinterleaved
layout and is merged on host.

Per batch iteration (H=512 output rows = 2 double-steps ds, each a pair
of even/odd row tiles E/O sharing the vertical row-pair reduction; 15
min/max elem-ops per pixel):
  per ds: one 12KB/partition input DMA (3 chunks; A/B chunk first),
          p=min(A,B), q=max(A,B)                        2 TT
          sort3 completion lo/md/hi (E/O stacked,
          stride-0 broadcast of p,q)                    3 TT
  then, stacked across ds*eo*C (strides nest C*3=eo, eo*2=ds, so the
  merged dim fits the 3-dim TensorTensor ISA limit):
          column pairs me_lo/mx_md/mn_md/me_hi          2 TT
          finals x,z,y interior (odd/even stacked)      4 TT
          boundary cols 0, W-1                          6 Act copies
          med3(x,y,z) -> res (batch 0 whole; the last
          batch in per-ds/per-eo pieces so final DMAs
          start early)                              7-10 TT
          half-column output DMAs

All min/max on the DVE: the Pool engine's software library rejects
TensorTensor min/max (walrus engine check) and an add/relu decomposition
is numerically unsafe (catastrophic cancellation vs max-rel-err).
Most tiles are bufs=1 — every producer/consumer is on the in-order DVE
queue — only the DMA-facing tiles (in4, res) are double-buffered.
"""

import sys

sys.path.insert(0, "/opt/trn_rl_repo")

import numpy as np
import ml_dtypes

_COMPILED = {}

B, C, H, W = 16, 3, 512, 512
NCORES = 8
BPC = B // NCORES
Wh = W // 2        # 256
HP = H + 2         # padded rows
SR = C * W         # elements per row (1536), same in deinterleaved layout
SBI = HP * SR      # input batch stride
SBO = H * SR       # output batch stride


def _legalize_waits(nc, mybir):
    """Hoist excess sync-waits into a preceding same-engine EventSemaphore.
    The TRN2 ISA allows 1 sync-wait on compute instructions (2 on DMACopy;
    EventSemaphore allows several) but Tile's scheduler can emit more."""
    limits = {"InstEventSemaphore": 2}
    n_hoisted = 0
    for f in nc.m.functions:
        for bb in f.blocks:
            il = bb.instructions
            idx = 0
            while idx < len(il):
                i = il[idx]
                si = i.sync_info
                lim = limits.get(type(i).__name__, 1)
                if si is not None and si.on_wait and len(si.on_wait) > lim:
                    waits = list(si.on_wait)
                    keep, excess = waits[:lim], waits[lim:]
                    hoists = []
                    for j in range(0, len(excess), 2):
                        h = mybir.InstEventSemaphore(
                            name=f"hoistw_{n_hoisted}", ins=[], outs=[])
                        n_hoisted += 1
                        h.engine = i.engine
                        h.sync_info = mybir.SyncInfo(
                            on_wait=excess[j:j + 2], on_update=[])
                        hoists.append(h)
                    i.sync_info = mybir.SyncInfo(
                        on_wait=keep, on_update=si.on_update)
                    for k, h in enumerate(hoists):
                        il.insert(idx + k, h)
                    idx += len(hoists)
                idx += 1
    return n_hoisted


def _build_nc():
    from concourse import bass
    import concourse.mybir as mybir
    from concourse.tile import TileContext

    bf16 = mybir.dt.bfloat16
    MIN = mybir.AluOpType.min
    MAX = mybir.AluOpType.max
    AP = bass.AP

    nc = bass.Bass()
    img = nc.dram_tensor("image", [BPC, HP, C, 2, Wh], bf16,
                         kind="ExternalInput")
    out = nc.dram_tensor("out", [BPC, H, C, 2, Wh], bf16,
                         kind="ExternalOutput")

    def mk(t, off, dims):
        b = t[:]
        return AP(b.tensor, b.offset + off, [list(b.ap[0])] + dims)

    with TileContext(nc) as tc:
        with tc.tile_pool(name="p", bufs=1) as pool:
            for g in range(BPC):
                base = g * SBI
                # tile layouts (per-partition element offsets):
                # in4 [2ds,4t,1536]  t: 0=cE 1=A 2=B 3=cO   off ds*6144+t*1536
                # P   [2ds,2pq,1536]                        off ds*3072+pq*1536
                # lmh [3s,2ds,2eo,C,2,Wh] s: lo,md,hi   off s*6144+ds*3072+eo*1536
                # hp  [4s,2ds,2eo,C,Wh] s: me_lo,mx_md,mn_md,me_hi
                #                                       off s*3072+ds*1536+eo*768
                # xyz [4s,2ds,2eo,C,2,Wh] s: x,y,z,f1   off s*6144+ds*3072+eo*1536
                # res [2ds,2eo,1536]
                in4 = pool.tile([128, 2, 4, C, 2, Wh], bf16, tag="in4", bufs=2)
                P = pool.tile([128, 2, 2, C, 2, Wh], bf16, tag="P")
                lmh = pool.tile([128, 3, 2, 2, C, 2, Wh], bf16, tag="lmh")
                hp = pool.tile([128, 4, 2, 2, C, Wh], bf16, tag="hp")
                xyz = pool.tile([128, 4, 2, 2, C, 2, Wh], bf16, tag="xyz")
                res = pool.tile([128, 2, 2, C, 2, Wh], bf16, tag="res", bufs=2)

                for ds in range(2):
                    r0 = 256 * ds
                    db = ds * 6144
                    # padded rows r0+2j+t per partition; A and B chunks first
                    # (separate queues transfer in parallel) so the P stage
                    # starts as early as possible
                    for t, dr in ((1, 1), (2, 2), (0, 0), (3, 3)):
                        nc.sync.dma_start(out=in4[:, ds, t], in_=AP(
                            img, base + (r0 + dr) * SR,
                            [[2 * SR, 128], [1, 1536]]))

                    # p = min(A,B), q = max(A,B)
                    nc.vector.tensor_tensor(
                        P[:, ds, 0], in4[:, ds, 1], in4[:, ds, 2], MIN)
                    nc.vector.tensor_tensor(
                        P[:, ds, 1], in4[:, ds, 1], in4[:, ds, 2], MAX)

                    # sort3 completion, E/O stacked:
                    # [loE,tE,loO,tO] = min([p,q,p,q],[cE,cE,cO,cO])
                    # [hiE,hiO] = max([q,q],[cE,cO]); md = max(p, t) in place
                    pb = ds * 3072
                    nc.vector.tensor_tensor(
                        mk(lmh, pb, [[1536, 2], [6144, 2], [1, 1536]]),
                        mk(P, pb, [[0, 2], [1536, 2], [1, 1536]]),
                        mk(in4, db, [[4608, 2], [0, 2], [1, 1536]]), MIN)
                    nc.vector.tensor_tensor(
                        mk(lmh, 2 * 6144 + pb, [[1536, 2], [1, 1536]]),
                        mk(P, pb + 1536, [[0, 2], [1, 1536]]),
                        mk(in4, db, [[4608, 2], [1, 1536]]), MAX)
                    md_ap = mk(lmh, 6144 + pb, [[1536, 2], [1, 1536]])
                    nc.vector.tensor_tensor(
                        md_ap, mk(P, pb, [[0, 2], [1, 1536]]), md_ap, MAX)

                # ---- column pairs me[j] = op(v[par0,j], v[par1,j]), stacked
                # over (slot pair, ds*eo*C): (lo,md)->MAX, (md,hi)->MIN
                nc.vector.tensor_tensor(
                    mk(hp, 0, [[3072, 2], [256, 12], [1, 256]]),
                    mk(lmh, 0, [[6144, 2], [512, 12], [1, 256]]),
                    mk(lmh, 256, [[6144, 2], [512, 12], [1, 256]]), MAX)
                nc.vector.tensor_tensor(
                    mk(hp, 2 * 3072, [[3072, 2], [256, 12], [1, 256]]),
                    mk(lmh, 6144, [[6144, 2], [512, 12], [1, 256]]),
                    mk(lmh, 6144 + 256, [[6144, 2], [512, 12], [1, 256]]), MIN)

                # ---- interior finals, odd/even output cols stacked:
                # i=0 -> odd col 2j+1 (par1,j), i=1 -> even col 2j+2
                # (par0,j+1), j=0..254
                o_ap = lambda s: mk(xyz, s * 6144 + 256,
                                    [[512, 12], [-255, 2], [1, 255]])
                me_ap = lambda s: mk(hp, s * 3072,
                                     [[256, 12], [1, 2], [1, 255]])
                th_ap = lambda s: mk(lmh, s * 6144 + 1,
                                     [[512, 12], [255, 2], [1, 255]])
                nc.vector.tensor_tensor(o_ap(0), me_ap(0), th_ap(0), MAX)
                nc.vector.tensor_tensor(o_ap(2), me_ap(3), th_ap(2), MIN)
                nc.vector.tensor_tensor(o_ap(1), me_ap(1), th_ap(1), MIN)
                nc.vector.tensor_tensor(o_ap(1), me_ap(2), o_ap(1), MAX)

                # ---- boundary columns (Act): col0 = even(par0,j0):
                # x=me_lo[0], z=me_hi[0], y=md[par1,0]; col W-1 =
                # odd(par1,j255): x=me_lo[255], z=me_hi[255], y=md[par0,255]
                for dst, src, st in ((0, 0, 0),
                                     (511, 255, 0),
                                     (2 * 6144, 3 * 3072, 0),
                                     (2 * 6144 + 511, 3 * 3072 + 255, 0),
                                     (6144, 6144 + 256, 1),
                                     (6144 + 511, 6144 + 255, 1)):
                    srct, sdim = ((hp, 256), (lmh, 512))[st]
                    nc.scalar.copy(mk(xyz, dst, [[512, 12], [1, 1]]),
                                   mk(srct, src, [[sdim, 12], [1, 1]]))

                # ---- med3(x,y,z) -> res. Batch 0's tail is hidden by batch
                # 1's front, so it uses the cheaper full-batch stacking; the
                # last batch finishes in small per-ds/per-eo pieces so the
                # final output DMAs start as early as possible.
                def med3(hb, n):
                    sl = lambda s: mk(xyz, s * 6144 + hb, [[1, n]])
                    f1, x, y, z = sl(3), sl(0), sl(1), sl(2)
                    nc.vector.tensor_tensor(f1, x, y, MIN)
                    nc.vector.tensor_tensor(x, x, y, MAX)
                    nc.vector.tensor_tensor(x, x, z, MIN)
                    return f1, x

                def res_out(hb, n, ds, eo):
                    for h in range(0, n, 768):
                        nc.sync.dma_start(
                            out=AP(out,
                                   g * SBO + (256 * ds + eo) * SR + h % 1536,
                                   [[2 * SR, 128], [1, 768]]),
                            in_=mk(res, hb + h, [[1, 768]]))

                if g == 0:
                    f1, x = med3(0, 6144)
                    nc.vector.tensor_tensor(
                        mk(res, 0, [[1, 6144]]), f1, x, MAX)
                    for ds in range(2):
                        for eo in range(2):
                            res_out(ds * 3072 + eo * 1536, 1536, ds, eo)
                else:
                    for ds in range(2):
                        hb = ds * 3072
                        f1, x = med3(hb, 3072)
                        for eo in range(2):
                            eb = hb + eo * 1536
                            nc.vector.tensor_tensor(
                                mk(res, eb, [[1, 1536]]),
                                mk(xyz, 3 * 6144 + eb, [[1, 1536]]),
                                mk(xyz, eb, [[1, 1536]]), MAX)
                            res_out(eb, 1536, ds, eo)

    _legalize_waits(nc, mybir)
    return nc


def _stage_input(img_k: np.ndarray) -> np.ndarray:
    """[BPC, C, H, W] f32 -> [BPC, HP, C, 2, Wh] bf16, rows padded
    (reflect), columns deinterleaved."""
    t = img_k.transpose(0, 2, 1, 3)  # [BPC, H, C, W]
    p = np.empty((BPC, HP, C, W), dtype=np.float32)
    p[:, 1:H + 1] = t
    p[:, 0] = t[:, 1]
    p[:, H + 1] = t[:, H - 2]
    q = np.empty((BPC, HP, C, 2, Wh), dtype=ml_dtypes.bfloat16)
    q[..., 0, :] = p[..., 0::2]
    q[..., 1, :] = p[..., 1::2]
    return q


def _unstage_output(o: np.ndarray) -> np.ndarray:
    """[BPC, H, C, 2, Wh] bf16 -> [BPC, C, H, W] f32."""
    full = np.empty((BPC, H, C, W), dtype=np.float32)
    full[..., 0::2] = o[..., 0, :]
    full[..., 1::2] = o[..., 1, :]
    return full.transpose(0, 2, 1, 3)


def kernel(image: np.ndarray) -> np.ndarray:
    from concourse.bass_utils import run_bass_kernel_spmd

    image = np.asarray(image, dtype=np.float32)
    if "nc" not in _COMPILED:
        _COMPILED["nc"] = _build_nc()
    nc = _COMPILED["nc"]

    in_maps = [{"image": _stage_input(image[k * BPC:(k + 1) * BPC])}
               for k in range(NCORES)]
    try:
        res = run_bass_kernel_spmd(nc, in_maps, core_ids=list(range(NCORES)))
    except Exception:
        res = run_bass_kernel_spmd(nc, in_maps, core_ids=list(range(NCORES)))
    return np.concatenate(
        [_unstage_output(np.asarray(res.results[k]["out"]))
         for k in range(NCORES)],
        axis=0)
